# revision 20
# baseline (speedup 1.0000x reference)
"""Trainium2 Bass kernel for nn_ButterflyFilter.

The reference applies, per length-512 row (flattened b*c*angles):
  zero-pad to 1024 -> 10-stage butterfly "FFT" (stage order decreasing)
  -> elementwise filter (bit-reversed order) -> 10-stage butterfly
  "IFFT" (stage order increasing) -> real part of first 512 entries.

Every step is linear in x, so the whole chain is one complex 1024x1024
operator A determined by (twiddle_fft, twiddle_ifft, fourier_filter_br).
Since x is real with support on [:512] and only Re(y)[:512] is kept, the
effective map is the real 512x512 matrix W = Re(A)[:512, :512]:

    proj_row = W @ x_row

x in HBM is (b, c, s, a) — for fixed (b, c) the tile is (s, a), i.e. rows
(angles) are already laid out column-major, exactly the moving-operand
layout the TensorEngine wants. So the device work is 16 independent
512x512x512 matmuls out_bc = W @ x_bc, data-parallel 2 per core across
8 cores. The small parameter-folding (building W from the twiddles) runs
on host in float64; the 32 MiB of row data never touches the host math.
"""

import os
import sys
import types
from contextlib import ExitStack

import numpy as np

import concourse.bass as bass
import concourse.mybir as mybir
from concourse.bass_utils import run_bass_kernel_spmd


def _ensure_axon_hooks():
    # concourse.bass_utils imports antenv.axon_hooks on the trace path; some
    # images lack that module. Provide a no-op holder so a BASS_TRACE env set
    # by the caller can't crash the run.
    try:
        import antenv.axon_hooks  # noqa: F401
    except Exception:
        m = types.ModuleType("antenv.axon_hooks")
        m._h = None
        m.set_axon_ntff_profile_hook = lambda h: setattr(m, "_h", h)
        m.get_axon_ntff_profile_hook = lambda: m._h
        sys.modules["antenv.axon_hooks"] = m


_ensure_axon_hooks()

N_CORES = 8
S = 512          # input/output row length
NF = 1024        # padded length
P = 128          # SBUF partitions
BC_PER_CORE = 2  # 16 (b,c) tiles / 8 cores

# Exposed for the test harness: exec time of the last device run (ns), if
# profiling was enabled via BUTTERFLY_TRACE=1.
last_exec_time_ns = None
last_results = None


def _butterfly_np(tw, x, increasing):
    # Mirrors the reference butterfly exactly, in numpy (any dtype).
    B, n = x.shape
    m = tw.shape[0]
    order = range(m) if increasing else range(m - 1, -1, -1)
    for idx in order:
        s = 1 << idx
        t = tw[idx].reshape(n // (2 * s), s, 2, 2)
        xr = x.reshape(B, n // (2 * s), 2, s)
        x = np.einsum('gjik,bgkj->bgij', t, xr).reshape(B, n)
    return x


def _compose_wt(twiddle_fft, twiddle_ifft, fourier_filter_br):
    """Fold twiddles+filter into the lhsT operand Wt[i_in, o_out] (512x512 f32)."""
    tw_fft = np.asarray(twiddle_fft, dtype=np.float64)
    tw_ifft = np.asarray(twiddle_ifft, dtype=np.float64)
    filt = np.asarray(fourier_filter_br, dtype=np.float64)
    tf = tw_fft[0, ..., 0] + 1j * tw_fft[0, ..., 1]
    ti = tw_ifft[0, ..., 0] + 1j * tw_ifft[0, ..., 1]
    X = np.eye(NF, dtype=np.complex128)      # row j = e_j
    X = _butterfly_np(tf, X, increasing=False)
    X = X * filt[None, :]
    X = _butterfly_np(ti, X, increasing=True)
    # X = chain(I) = A^T, so X[i, o] = A[o, i]; W[o, i] = Re(A[o, i]).
    # lhsT for out = lhsT.T @ rhs must be Wt[i, o] = W[o, i] = Re(X[i, o]).
    return np.ascontiguousarray(np.real(X[:S, :S]).astype(np.float32))


def _mm_dtype():
    return (
        mybir.dt.float32r
        if os.environ.get("BUTTERFLY_MM_DTYPE", "fp32r") == "fp32r"
        else mybir.dt.float32
    )


def _build_nc():
    # Raw Bass (no TileContext): this walrus encodes at most ONE semaphore
    # wait per instruction, which Tile's scheduler and epilogue drain cannot
    # guarantee. With manual engine programs every wait is its own wait_ge.
    #
    # Layout (per core):
    #   wx[j] (128, 2048) = [W_{2j} | x0_{2j} | W_{2j+1} | x0_{2j+1}] so each
    #   matmul's lhsT and rhs live in one DMA'd tile. x1 is the second bc
    #   tile's rows packed (128, 2048). out_bc[o*128+p, a] lands in an SBUF
    #   (128, 2048) tile per bc, stored with one strided DMA each.
    mmdt = _mm_dtype()
    kc = S // P  # 4 contraction chunks
    oc = S // P  # 4 output-row chunks
    f32 = mybir.dt.float32
    # PE warm-up matmuls (HAM un-throttle) during the input DMA wait. Each
    # fp32 matmul emits 2 HW passes at ~640 ns cold, so 3 calls ~= 3.8 us of
    # dense PE busy — enough to trip HAM's ~3.4 us SHORT window right as the
    # first input piece lands (measured: 2 calls leave the real stream cold).
    n_warm = 3

    nc = bass.Bass()
    wx = nc.declare_dram_parameter("wx", [kc, P, 2 * S], mmdt, isOutput=False)
    x1d = nc.declare_dram_parameter("x1", [kc, P, S], mmdt, isOutput=False)
    out = nc.declare_dram_parameter("out", [BC_PER_CORE, S, S], f32, isOutput=True)

    with ExitStack() as ctx:
        wx_sb = [
            ctx.enter_context(nc.sbuf_tensor(f"wx_sb{k}", [P, 2 * S], mmdt))
            for k in range(kc)
        ]
        x1_sb = ctx.enter_context(nc.sbuf_tensor("x1_sb", [P, 4 * S], mmdt))
        warm_sb = ctx.enter_context(nc.sbuf_tensor("warm_sb", [P, 3 * P], f32))
        o_sb = [
            ctx.enter_context(nc.sbuf_tensor(f"o_sb{j}", [P, 4 * S], f32))
            for j in range(2)
        ]
        accs = [
            ctx.enter_context(nc.psum_tensor(f"acc{g}", [P, S], f32))
            for g in range(BC_PER_CORE * oc)
        ]
        s_wx = [ctx.enter_context(nc.semaphore(f"s_wx{k}")) for k in range(kc)]
        s_x1 = [ctx.enter_context(nc.semaphore(f"s_x1{k}")) for k in range(kc)]
        s_warm = ctx.enter_context(nc.semaphore("s_warm"))
        s_pe = ctx.enter_context(nc.semaphore("s_pe"))
        s_dve = ctx.enter_context(nc.semaphore("s_dve"))
        s_out = ctx.enter_context(nc.semaphore("s_out"))
        block = ctx.enter_context(nc.Block())

        @block.sync
        def _(sync):
            # Input pieces, issue order = consumption order. 512 KiB each for
            # wx (W chunk fused with bc0 x chunk), 256 KiB each for x1.
            for k in range(kc):
                sync.dma_start(wx_sb[k][:], wx[k]).then_inc(s_wx[k], 16)
            for k in range(kc):
                sync.dma_start(x1_sb[:, bass.ts(k, S)], x1d[k]).then_inc(s_x1[k], 16)
            sync.wait_ge(s_out, BC_PER_CORE * oc * 16)

        @block.tensor
        def _(tensor):
            # Warm-up matmuls on a zeroed scratch tile: keeps the PE busy
            # while inputs stream in so HAM un-throttles (1.2 -> 2.4 GHz)
            # before the real matmuls. Results land in acc 7 which is cleared
            # by its real accumulation group's start=True much later.
            tensor.wait_ge(s_warm, 1)
            for _ in range(n_warm):
                nc.tensor.matmul(
                    accs[-1][:, : 2 * P], warm_sb[:, :P], warm_sb[:, P : 3 * P],
                    start=True, stop=True,
                )
            # bc0: k-outer so compute starts on the first 512 KiB piece.
            for k in range(kc):
                tensor.wait_ge(s_wx[k], 16)
                for o in range(oc):
                    mm = nc.tensor.matmul(
                        accs[o][:],
                        wx_sb[k][:, bass.ts(o, P)],
                        wx_sb[k][:, S : 2 * S],
                        start=(k == 0),
                        stop=(k == kc - 1),
                    )
                    if k == kc - 1:
                        mm.then_inc(s_pe, 1)
            # bc1
            for k in range(kc):
                tensor.wait_ge(s_x1[k], 16)
                for o in range(oc):
                    mm = nc.tensor.matmul(
                        accs[oc + o][:],
                        wx_sb[k][:, bass.ts(o, P)],
                        x1_sb[:, bass.ts(k, S)],
                        start=(k == 0),
                        stop=(k == kc - 1),
                    )
                    if k == kc - 1:
                        mm.then_inc(s_pe, 1)

        @block.vector
        def _(vector):
            nc.vector.memset(warm_sb[:], 0.0).then_inc(s_warm, 1)
            for g in range(BC_PER_CORE * oc):
                bc, o = divmod(g, oc)
                vector.wait_ge(s_pe, g + 1)
                nc.vector.tensor_copy(
                    o_sb[bc][:, bass.ts(o, S)], accs[g][:]
                ).then_inc(s_dve, 1)

        @block.scalar
        def _(scalar):
            # Per-group 256 KiB stores from the otherwise-idle ACT engine so
            # output drains as soon as each o-chunk is copied out of PSUM.
            # Hold stores until the last input piece is in: the input tail
            # (x1 pieces) gates the final matmuls, so giving it the full HBM
            # pipe beats overlapping slack-rich output traffic with it.
            scalar.wait_ge(s_x1[kc - 1], 16)
            for g in range(BC_PER_CORE * oc):
                bc, o = divmod(g, oc)
                scalar.wait_ge(s_dve, g + 1)
                scalar.dma_start(
                    out[bc, bass.ts(o, P), :], o_sb[bc][:, bass.ts(o, S)]
                ).then_inc(s_out, 16)

    return nc


def kernel(x, twiddle_fft, twiddle_ifft, fourier_filter_br):
    global last_exec_time_ns, last_results
    x = np.asarray(x, dtype=np.float32)
    b, c, s_len, a = x.shape
    assert (b, c, s_len, a) == (8, 2, S, S)

    wt = _compose_wt(twiddle_fft, twiddle_ifft, fourier_filter_br)
    x16 = x.reshape(b * c, S // P, P, S)  # [bc, k, p, m]
    wt4 = wt.reshape(S // P, P, S)

    in_maps = []
    for core in range(N_CORES):
        x0 = x16[BC_PER_CORE * core]
        x1 = x16[BC_PER_CORE * core + 1]
        # wx[k] = [w_k | x0_k] along the free dim, one 512 KiB DMA piece each
        wx = np.concatenate([wt4, x0], axis=2)  # (4, 128, 1024)
        in_maps.append(
            {
                "wx": np.ascontiguousarray(wx),
                "x1": np.ascontiguousarray(x1),
            }
        )
    nc = _build_nc()
    trace = os.environ.get("BUTTERFLY_TRACE") == "1"
    res = run_bass_kernel_spmd(nc, in_maps, core_ids=list(range(N_CORES)), trace=trace)
    last_exec_time_ns = res.exec_time_ns
    last_results = res

    q = np.concatenate([res.results[k]["out"] for k in range(N_CORES)], axis=0)
    # q[bc, o, a] = proj.T[o, bc*512 + a]; reference output is
    # proj.T.reshape(b, c, s, a) — a pure reinterpret of the (512, 8192) buffer.
    out = q.transpose(1, 0, 2).reshape(S, b * c * a).reshape(b, c, s_len, a)
    return np.ascontiguousarray(out).astype(np.float32)
